# revision 4
# baseline (speedup 1.0000x reference)
import sys

sys.path.insert(0, "/opt/trn_rl_repo")

import numpy as np
import ml_dtypes
from contextlib import ExitStack

# Problem constants (hardcoded per contract: kernel.py is self-contained).
B, S, D, O, M, E = 8, 2048, 768, 512, 1536, 8
T = S  # tokens per core (data-parallel over batch: 1 batch row per core)
P = 128
DT = D // P   # 6 d-tiles
MT = M // P   # 12 m-tiles
NT = T // P   # 16 token tiles per core
NCORES = 8

SX = 16.0     # fp8 scale for x
SW = 1024.0   # fp8 scale for W_in
E4NP = ml_dtypes.float8_e4m3
BFNP = ml_dtypes.bfloat16

_CACHE = {}


def _build():
    import concourse.bass as bass
    import concourse.tile as tile
    from concourse import bacc, mybir
    from concourse.masks import make_identity

    f32 = mybir.dt.float32
    bf16 = mybir.dt.bfloat16
    fp8 = mybir.dt.float8e4
    AF = mybir.ActivationFunctionType
    ALU = mybir.AluOpType
    DR = mybir.MatmulPerfMode.DoubleRow

    nc = bacc.Bacc("TRN2", target_bir_lowering=False, debug=False,
                   num_devices=NCORES)

    # All operands arrive pre-transposed / pre-cast from the host.
    xt8_d = nc.dram_tensor("xt8", (D, T), fp8, kind="ExternalInput").ap()
    xtb_d = nc.dram_tensor("xtb", (D, T), bf16, kind="ExternalInput").ap()
    wg_d = nc.dram_tensor("wg", (P, DT, E), bf16, kind="ExternalInput").ap()
    win_d = nc.dram_tensor("win8", (E, D, M), fp8, kind="ExternalInput").ap()
    wout_d = nc.dram_tensor("woutT", (E, M, O), bf16, kind="ExternalInput").ap()
    wsc_d = nc.dram_tensor("wscT", (E, D, O), bf16, kind="ExternalInput").ap()
    negc_d = nc.dram_tensor("negcT", (P, MT, E), f32, kind="ExternalInput").ap()
    bo_d = nc.dram_tensor("bo", (E, O), bf16, kind="ExternalInput").ap()
    out_d = nc.dram_tensor("out", (T, O), f32, kind="ExternalOutput").ap()

    with tile.TileContext(nc) as tc, ExitStack() as ctx:
        const = ctx.enter_context(tc.tile_pool(name="const", bufs=1))
        wp = ctx.enter_context(tc.tile_pool(name="wp", bufs=2))
        comb = ctx.enter_context(tc.tile_pool(name="comb", bufs=2))
        pmm1 = ctx.enter_context(tc.tile_pool(name="pmm1", bufs=5, space="PSUM"))
        pmm2 = ctx.enter_context(tc.tile_pool(name="pmm2", bufs=2, space="PSUM"))
        ptr = ctx.enter_context(tc.tile_pool(name="ptr", bufs=1, space="PSUM"))

        ident = const.tile([P, P], bf16)
        make_identity(nc, ident)

        # ---- persistent SBUF tensors ----
        xT8 = const.tile([P, DT, T], fp8)        # 16*x^T  [d_in, dt, t]
        xT = const.tile([P, DT, T], bf16)        # x^T
        hT = const.tile([P, MT, T], bf16)        # gelu output, full T
        acc = const.tile([P, NT, O], f32)        # output accumulator
        g_exp = const.tile([P, NT, E], f32)      # unnormalized softmax numerators
        g_bf = const.tile([P, NT, E], bf16)
        rinv = const.tile([P, NT], f32)          # 1 / sum_e exp
        gsum = const.tile([P, NT], f32)
        gTexp = const.tile([P, NT, P], bf16)     # gates transposed [e<=8, tt, t]
        wgate = const.tile([P, DT, E], bf16)
        negcT = const.tile([P, MT, E], f32)      # [m_in, mt, e]
        bo_sb = const.tile([P, O], bf16)

        def load_weights(e):
            winT8 = wp.tile([P, DT, M], fp8, tag="win")
            woutT = wp.tile([P, MT, O], bf16, tag="wout")
            wscT = wp.tile([P, DT, O], bf16, tag="wsc")
            nc.sync.dma_start(winT8, win_d[e].rearrange("(dt p) m -> p dt m", p=P))
            nc.sync.dma_start(woutT, wout_d[e].rearrange("(mt p) o -> p mt o", p=P))
            nc.sync.dma_start(wscT, wsc_d[e].rearrange("(dt p) o -> p dt o", p=P))
            return winT8, woutT, wscT

        # ---- input DMAs, ordered so e0 mm1 can start earliest ----
        nc.sync.dma_start(negcT, negc_d)
        nc.sync.dma_start(xT8, xt8_d.rearrange("(dt p) t -> p dt t", p=P))
        w0 = load_weights(0)
        nc.sync.dma_start(wgate, wg_d)
        nc.sync.dma_start(xT, xtb_d.rearrange("(dt p) t -> p dt t", p=P))
        nc.sync.dma_start(bo_sb[:E, :], bo_d)

        def mm1_q(e, winT8, tq):
            """h^T for 512 tokens: 12 m-tiles, 3 DoubleRow matmuls each."""
            t0 = tq * 512
            for mt in range(MT):
                ph = pmm1.tile([P, O], f32, tag="mm1")
                for k in range(3):
                    nc.tensor.matmul(
                        ph, winT8[:, 2 * k:2 * k + 2, mt * P:(mt + 1) * P],
                        xT8[:, 2 * k:2 * k + 2, t0:t0 + 512],
                        start=(k == 0), stop=(k == 2), perf_mode=DR)
                nc.scalar.activation(hT[:, mt, t0:t0 + 512], ph, AF.Gelu,
                                     bias=negcT[:, mt, e:e + 1],
                                     scale=1.0 / (SX * SW))

        def mm2_t(e, woutT, wscT, tg):
            """one [128-token, 512] output tile: h@W_out^T + x@W_sc^T."""
            po = pmm2.tile([P, O], f32, tag="mm2")
            for mt in range(MT):
                nc.tensor.matmul(po, hT[:, mt, tg * P:(tg + 1) * P],
                                 woutT[:, mt, :],
                                 start=(mt == 0), stop=False)
            for dt_ in range(DT):
                nc.tensor.matmul(po, xT[:, dt_, tg * P:(tg + 1) * P],
                                 wscT[:, dt_, :],
                                 start=False, stop=(dt_ == DT - 1))
            tmp = comb.tile([P, O], f32, tag="tmp")
            nc.vector.tensor_scalar(out=tmp, in0=po,
                                    scalar1=g_exp[:, tg, e:e + 1],
                                    scalar2=rinv[:, tg:tg + 1],
                                    op0=ALU.mult, op1=ALU.mult)
            nc.gpsimd.tensor_add(acc[:, tg, :], acc[:, tg, :], tmp)
            if e == E - 1:
                nc.scalar.dma_start(out_d[tg * P:(tg + 1) * P, :], acc[:, tg, :])

        # ---- expert 0 mm1 first (needs only xT8 + win8[0]) ----
        mm1_q(0, w0[0], 0)
        mm1_q(0, w0[0], 1)

        # ---- gating (needs xT + wgate) ----
        for tt in range(NT):
            pg = pmm2.tile([P, E], f32, tag="mm2")
            for dt_ in range(DT):
                nc.tensor.matmul(pg, xT[:, dt_, tt * P:(tt + 1) * P],
                                 wgate[:, dt_, :],
                                 start=(dt_ == 0), stop=(dt_ == DT - 1))
            nc.scalar.activation(g_exp[:, tt, :], pg, AF.Exp)

        mm1_q(0, w0[0], 2)

        nc.vector.tensor_reduce(gsum, g_exp, axis=mybir.AxisListType.X, op=ALU.add)
        nc.vector.reciprocal(rinv, gsum)
        nc.gpsimd.tensor_copy(g_bf, g_exp)

        # transpose gates ([128,8] blocks -> [8,128]) for the b_out init matmul
        for g in range(4):
            pt = ptr.tile([P, 4, P], bf16, tag="gtr")
            for i in range(4):
                tt = g * 4 + i
                nc.tensor.transpose(pt[:E, i, :], g_bf[:, tt, :], ident)
            nc.vector.tensor_copy(gTexp[:E, g * 4:(g + 1) * 4, :], pt[:E, :4, :])

        mm1_q(0, w0[0], 3)

        # acc init: acc[t, o] = (g_exp[t, :] @ b_out) * rinv[t]
        for tt in range(NT):
            pb = pmm2.tile([P, O], f32, tag="mm2")
            nc.tensor.matmul(pb, gTexp[:E, tt, :], bo_sb[:E, :])
            nc.vector.tensor_scalar_mul(acc[:, tt, :], pb,
                                        scalar1=rinv[:, tt:tt + 1])

        # ---- expert pipeline ----
        winT8, woutT, wscT = w0
        for e in range(E):
            if e + 1 < E:
                nw = load_weights(e + 1)
            if e == 0:
                for tg in range(NT):
                    mm2_t(e, woutT, wscT, tg)
            else:
                # interleave so mm2 never waits on the gelu of its own tokens
                mm1_q(e, winT8, 0)
                mm1_q(e, winT8, 1)
                for tg in range(4):
                    mm2_t(e, woutT, wscT, tg)
                mm1_q(e, winT8, 2)
                for tg in range(4, 8):
                    mm2_t(e, woutT, wscT, tg)
                mm1_q(e, winT8, 3)
                for tg in range(8, NT):
                    mm2_t(e, woutT, wscT, tg)
            if e + 1 < E:
                winT8, woutT, wscT = nw

    nc.compile()
    return nc


def _get_nc():
    if "nc" not in _CACHE:
        _CACHE["nc"] = _build()
    return _CACHE["nc"]


def _q8(a, scale):
    return np.clip(np.asarray(a, np.float32) * scale, -240, 240).astype(E4NP)


def prep_in_maps(x, w_gate, bias_in, W_in, W_out, b_out, W_sc):
    x = np.asarray(x, np.float32)
    W_in = np.asarray(W_in, np.float32)
    negc = -np.einsum("ed,emd->em", np.asarray(bias_in, np.float64),
                      np.asarray(W_in, np.float64)).astype(np.float32)
    negcT = np.ascontiguousarray(negc.T.reshape(MT, P, E).transpose(1, 0, 2))
    wg = np.ascontiguousarray(
        np.asarray(w_gate, np.float32).reshape(DT, P, E).transpose(1, 0, 2)
    ).astype(BFNP)
    shared = {
        "wg": wg,
        "negcT": negcT,
        "win8": np.ascontiguousarray(
            _q8(W_in.transpose(0, 2, 1), SW)),
        "woutT": np.ascontiguousarray(
            np.asarray(W_out, np.float32).transpose(0, 2, 1).astype(BFNP)),
        "wscT": np.ascontiguousarray(
            np.asarray(W_sc, np.float32).transpose(0, 2, 1).astype(BFNP)),
        "bo": np.ascontiguousarray(np.asarray(b_out, np.float32).astype(BFNP)),
    }
    in_maps = []
    for i in range(NCORES):
        xt = np.ascontiguousarray(x[i].T)
        in_maps.append({
            "xt8": _q8(xt, SX),
            "xtb": xt.astype(BFNP),
            **shared,
        })
    return in_maps


def kernel(x, w_gate, bias_in, W_in, W_out, b_out, W_sc):
    from concourse.bass_utils import run_bass_kernel_spmd

    nc = _get_nc()
    in_maps = prep_in_maps(x, w_gate, bias_in, W_in, W_out, b_out, W_sc)
    res = run_bass_kernel_spmd(nc, in_maps, core_ids=list(range(NCORES)))
    out = np.stack([res.results[i]["out"] for i in range(NCORES)], axis=0)
    return out.astype(np.float32)
